# revision 26
# baseline (speedup 1.0000x reference)
"""Trainium2 Bass kernel for nn_MultiHeadAttention (fused QKV + RMS-norm +
RoPE + masked softmax attention + out-proj), tensor-parallel over heads
across 8 NeuronCores.

Contract: kernel(**inputs) takes FULL inputs, returns FULL output.
Self-contained: imports only numpy + the concourse framework.
"""
import numpy as np

import concourse.bacc as bacc
import concourse.mybir as mybir
import concourse.tile as tile
from concourse import bass_utils

N_HEAD = 16
ROPE_BASE = 100000.0
RMS_EPS = 1e-5
L = 2048
D = 2048
HD = 128          # head dim
N_CORES = 8
HPD = N_HEAD // N_CORES   # heads per device = 2
QT = 512          # q tile (attention + l-tile width)
NQT = L // QT     # 4
NKB = L // 128    # 16 k-blocks
NEG = -1.0e9
SCALE = 1.0 / np.sqrt(HD)
EXP_BIAS = -0.7   # keeps exp() outputs < fp16 max (score*SCALE <= 11.32)

F32 = mybir.dt.float32
F32R = mybir.dt.float32r
F16 = mybir.dt.float16

_prog_cache: dict = {}


def _classify_mask(bm: np.ndarray):
    """Per (q-tile, k-block) classification of the boolean mask.

    plan[qt][kb] = ('full',) | ('skip',) | ('part', bias_idx, bstart, bw,
    zstart, zw): add biases[bias_idx][:, :bw] to score cols
    [bstart:bstart+bw], add NEG to cols [zstart:zstart+zw]. biases is
    [n, 128, 512] fp32 (col-sliced bias subtiles, zero-padded)."""
    plan = []
    uniq = {}
    biases = []
    for qt in range(NQT):
        row = []
        for kb in range(NKB):
            reg = bm[qt * QT:(qt + 1) * QT, kb * 128:(kb + 1) * 128]
            if reg.all():
                row.append(("full",))
                continue
            if not reg.any():
                row.append(("skip",))
                continue
            regT = reg.T  # [128 k, 512 q]
            col_all = regT.all(axis=0)       # fully open columns
            col_none = (~regT).all(axis=0)   # fully masked columns
            mixed = ~(col_all | col_none)
            mix_idx = np.nonzero(mixed)[0]
            none_idx = np.nonzero(col_none)[0]
            # contiguity of ranges (true for causal masks)
            ok = True
            bs = bw = zs = zw = 0
            if mix_idx.size:
                bs, be = int(mix_idx[0]), int(mix_idx[-1]) + 1
                bw = be - bs
                ok &= bool(mixed[bs:be].all())
            if none_idx.size:
                zs, ze = int(none_idx[0]), int(none_idx[-1]) + 1
                zw = ze - zs
                ok &= bool(col_none[zs:ze].all())
                ok &= not (bw and not (ze <= bs or be <= zs))
            if not ok:
                # fallback: full-width bias
                bs, bw, zs, zw = 0, QT, 0, 0
            if bw:
                bias = np.zeros((128, QT), np.float32)
                bias[:, 0:bw] = np.where(regT[:, bs:bs + bw], np.float32(0),
                                         np.float32(NEG))
                key = (bw, bias.tobytes())
                if key not in uniq:
                    uniq[key] = len(biases)
                    biases.append(bias)
                bi = uniq[key]
            else:
                bi = 0
            row.append(("part", bi, bs, bw, zs, zw))
        plan.append(tuple(row))
    if not biases:
        biases.append(np.zeros((128, QT), np.float32))
    return tuple(plan), np.stack(biases)


def _narrow(ent):
    """Column range start + NEG fallback region for a plan entry."""
    if ent[0] != "part":
        return 0, None
    _, bi, bs, bw, zs, zw = ent
    if zw and zs == 0:
        return zw, None           # masked prefix: just skip those columns
    if zw:
        return 0, (zs, zw)        # masked suffix (non-causal fallback)
    return 0, None


def _stagger(blocks, tail):
    """Software-pipeline: emit A(n+1) before B(n) so the in-order PE
    stream never waits on block n's exp before issuing block n+1's MM."""
    seq = []
    prevB = None
    for A, B in blocks:
        seq.append(A)
        if prevB is not None:
            seq.append(prevB)
        prevB = B
    if prevB is not None:
        seq.append(prevB)
    seq.extend(tail)
    return seq


def _weave(fillers, work):
    """Interleave filler units (dense independent PE work) between work
    units so dependency stalls in `work` are hidden."""
    if not work:
        for f in fillers:
            f()
        return
    if not fillers:
        for w in work:
            w()
        return
    per = len(fillers) / len(work)
    fi = 0.0
    done = 0
    for w in work:
        tgt = fi + per
        while done < min(len(fillers), int(tgt + 0.5)):
            fillers[done]()
            done += 1
        fi = tgt
        w()
    while done < len(fillers):
        fillers[done]()
        done += 1


def _build_program(plan, n_bias):
    nc = bacc.Bacc("TRN2", target_bir_lowering=False, debug=False)

    # ---- DRAM I/O (host pre-arranged for wide DMA lines) ----
    # xL: per l-tile, 16 d-slices of [128, 512] concatenated along free dim.
    xL_d = nc.dram_tensor("xL", [128, NQT * 16 * QT], F16, kind="ExternalInput")
    # wqkH: 16 d-slices of [128, 512] (cols: q_h0 | k_h0 | q_h1 | k_h1)
    wqk_d = nc.dram_tensor("wqkH", [128, 16 * 512], F16, kind="ExternalInput")
    wv_d = nc.dram_tensor("wvH", [128, 16 * 256], F16, kind="ExternalInput")
    wout_d = nc.dram_tensor("woutH", [128, 2 * D], F16, kind="ExternalInput")
    # cosD/sinD: per l-tile slice duplicated (for paired q|k rope ops)
    cos_d = nc.dram_tensor("cosD", [64, 2 * L], F16, kind="ExternalInput")
    sin_d = nc.dram_tensor("sinD", [64, 2 * L], F16, kind="ExternalInput")
    ones16_d = nc.dram_tensor("ones16", [128, 1], F16, kind="ExternalInput")
    ones_d = nc.dram_tensor("ones128", [128, 1], F32R, kind="ExternalInput")
    bias_d = nc.dram_tensor("biasT", [n_bias, 128, QT], F32, kind="ExternalInput")
    out_d = nc.dram_tensor("out", [L, D], F16, kind="ExternalOutput")

    Exp = mybir.ActivationFunctionType.Exp
    Ln = mybir.ActivationFunctionType.Ln

    with nc.allow_low_precision(reason="fp16 activations; fp32 accumulation"), \
         tile.TileContext(nc) as tc:
        with (
            tc.tile_pool(name="const", bufs=1) as cpool,
            tc.tile_pool(name="act", bufs=1) as apool,
            tc.tile_pool(name="wrk", bufs=1) as wpool,
            tc.tile_pool(name="ps", bufs=1, space="PSUM") as ps,
        ):
            # ---- tiny consts first ----
            ones16 = cpool.tile([128, 1], F16, name="ones16", tag="ones16")
            nc.sync.dma_start(ones16[:], ones16_d.ap())
            epsc = cpool.tile([128, 1], F32, name="epsc", tag="epsc")
            nc.vector.memset(epsc[:], RMS_EPS)
            ebc = cpool.tile([128, 1], F32, name="ebc", tag="ebc")
            nc.vector.memset(ebc[:], EXP_BIAS)
            ones128 = cpool.tile([128, 1], F32R, name="ones128", tag="ones128")
            nc.sync.dma_start(ones128[:], ones_d.ap())

            # ---- bulk inputs: ALL issued before any output DMA so the
            # in-order sync queue never blocks a later input behind an
            # output waiting on compute. l-tile 0 x/weights interleaved in
            # 4-slice chunks so the first projection chains start early.
            xbig = [cpool.tile([128, 16 * QT], F16, name=f"xb{lt}",
                               tag=f"xb{lt}") for lt in range(NQT)]
            wqk_all = cpool.tile([128, 16 * 512], F16, name="wqk_all",
                                 tag="wqk_all")
            CH = 4 * QT  # 4-slice chunk
            for c in range(4):
                nc.sync.dma_start(xbig[0][:, c * CH:(c + 1) * CH],
                                  xL_d.ap()[:, c * CH:(c + 1) * CH])
                nc.sync.dma_start(wqk_all[:, c * 2048:(c + 1) * 2048],
                                  wqk_d.ap()[:, c * 2048:(c + 1) * 2048])
            cosD = cpool.tile([64, 2 * L], F16, name="cosD", tag="cosD")
            sinD = cpool.tile([64, 2 * L], F16, name="sinD", tag="sinD")
            nc.sync.dma_start(cosD[:], cos_d.ap())
            nc.sync.dma_start(sinD[:], sin_d.ap())
            nc.sync.dma_start(xbig[1][:], xL_d.ap()[:, 8192:2 * 8192])
            wv_all = cpool.tile([128, 16 * 256], F16, name="wv_all",
                                tag="wv_all")
            nc.sync.dma_start(wv_all[:], wv_d.ap())
            btiles = []
            for b in range(n_bias):
                bt = cpool.tile([128, QT], F32, name=f"bias{b}", tag=f"bias{b}")
                nc.sync.dma_start(bt[:], bias_d.ap()[b])
                btiles.append(bt)
            wout_all = cpool.tile([128, 2 * D], F16, name="wout_all",
                                  tag="wout_all")
            nc.sync.dma_start(wout_all[:], wout_d.ap())
            nc.sync.dma_start(xbig[2][:], xL_d.ap()[:, 2 * 8192:3 * 8192])
            nc.sync.dma_start(xbig[3][:], xL_d.ap()[:, 3 * 8192:4 * 8192])
            # Delay then trigger the ~9us gpsimd library load: a plain DMA
            # (no library needed) makes the in-order gpsimd engine wait for
            # l-tile 0's chunk-2 before it reaches the LOAD_LIB that the
            # framework inserts ahead of the first custom op (the dummy
            # broadcast).  The load then overlaps l-tile-0 compute instead
            # of stealing DMA bandwidth from the startup-critical fill.
            db_in = cpool.tile([1, 16], F16, name="db_in", tag="db_in")
            nc.gpsimd.dma_start(db_in[:], xbig[0][0:1, 6128:6144])
            db_out = cpool.tile([128, 16], F16, name="db_out", tag="db_out")
            nc.gpsimd.partition_broadcast(db_out[:], db_in[:])

            # ---- persistent activations ----
            # qk_rot[lt][h]: [128, 1024] fp16, cols 0:512 = q (rope+rms),
            # cols 512:1024 = k (rope+rms) -> doubles as the k cache.
            qk_rot = [[apool.tile([128, 1024], F16, name=f"qkr{lt}_{h}",
                                  tag=f"qkr{lt}_{h}") for h in range(HPD)]
                      for lt in range(NQT)]
            aot = [apool.tile([128, L], F16, name=f"aot{h}", tag=f"aot{h}")
                   for h in range(HPD)]
            vnat = [apool.tile([128, 2 * HD], F16, name=f"vnat{lb}",
                               tag=f"vnat{lb}") for lb in range(16)]

            # attention state per (qt, h), held across emission groups
            astate = {}

            def emit_chains(lt):
                """q/k projection chains -> filler unit closures (4 MMs each)
                plus trailing raw/sq/ssq/rms/rope emission attached."""
                xb = xbig[lt]
                s1s = [None] * 4
                ssqs = [None] * 4
                rawp = [wpool.tile([128, 1024], F16, name="rawp", tag="rawp",
                                   bufs=4) for _ in range(2)]
                pps = []
                if lt == 0:
                    for ob in range(4):
                        tag, nb = (("pqk", 3) if ob < 2 else ("sT", 3))
                        pps.append(ps.tile([128, QT], F32, name=f"pqk0_{ob}",
                                           tag=tag, bufs=nb))
                else:
                    pps = [None] * 4

                units = []
                if lt == 0:
                    # DMA-inflow-bound: i-outer over 4 concurrent chains
                    def mk0(i):
                        def u():
                            for ob in range(4):
                                nc.tensor.matmul(
                                    pps[ob][:],
                                    wqk_all[:, i * 512 + ob * 128:
                                            i * 512 + (ob + 1) * 128],
                                    xb[:, i * QT:(i + 1) * QT],
                                    start=(i == 0), stop=(i == 15),
                                )
                            if i == 15:
                                for ob in range(4):
                                    _chain_tail(ob)
                        return u
                    units = [mk0(i) for i in range(16)]
                else:
                    def mkc(ob, ig):
                        def u():
                            if ig == 0:
                                pps[ob] = ps.tile(
                                    [128, QT], F32, name=f"pqk{lt}_{ob}",
                                    tag="pqk", bufs=3)
                            for i in range(4 * ig, 4 * ig + 4):
                                nc.tensor.matmul(
                                    pps[ob][:],
                                    wqk_all[:, i * 512 + ob * 128:
                                            i * 512 + (ob + 1) * 128],
                                    xb[:, i * QT:(i + 1) * QT],
                                    start=(i == 0), stop=(i == 15),
                                )
                            if ig == 3:
                                _chain_tail(ob)
                        return u
                    units = [mkc(ob, ig) for ob in range(4) for ig in range(4)]

                def _chain_tail(ob):
                    h, isk = ob // 2, ob % 2
                    raw = rawp[h][:, isk * QT:(isk + 1) * QT]
                    nc.vector.tensor_copy(out=raw, in_=pps[ob][:])
                    sq = wpool.tile([128, QT], F16, name="sq", tag="sq",
                                    bufs=2)
                    nc.vector.tensor_mul(out=sq[:], in0=raw, in1=raw)
                    ssq = ps.tile([128, QT], F32, name=f"ssq{lt}_{ob}",
                                  tag="sT", bufs=3)
                    nc.tensor.matmul(ssq[0:1, :], ones16[:], sq[:],
                                     start=True, stop=True)
                    ssqs[ob] = ssq

                def emit_rope():
                    ls = lt * QT
                    # rms scales s = exp(-0.5*ln(ssq/HD+eps)) = rsqrt(mean).
                    # All Ln's then all Exp's: Ln and Exp live in different
                    # ACT table sets, so interleaving them (or weaving
                    # attention exps between them) thrashes the 1.3us table
                    # load ~7x per l-tile; batching costs exactly 2 swaps.
                    lnms = []
                    for ob in range(4):
                        lnm = wpool.tile([1, QT], F16, name="lnm", tag="lnm",
                                         bufs=4)
                        nc.scalar.activation(lnm[:], ssqs[ob][0:1, :], Ln,
                                             bias=epsc[0:1, :],
                                             scale=1.0 / HD)
                        lnms.append(lnm)
                    for ob in range(4):
                        s1 = wpool.tile([1, QT], F16, name="s1", tag="s1",
                                        bufs=4)
                        nc.scalar.activation(s1[:], lnms[ob][:], Exp,
                                             scale=-0.5)
                        s1s[ob] = s1
                    s2ps = []
                    for h in range(2):
                        s2p = wpool.tile([128, 1024], F16, name="s2p",
                                         tag="s2p", bufs=2)
                        nc.gpsimd.partition_broadcast(s2p[:, 0:QT],
                                                      s1s[2 * h][:])
                        nc.gpsimd.partition_broadcast(s2p[:, QT:1024],
                                                      s1s[2 * h + 1][:])
                        s2ps.append(s2p)
                    # rope cores for both heads first (no s2p dependency),
                    # then the two rms-scale muls (which wait on gpsimd)
                    for h in range(2):
                        rw = rawp[h]
                        dst = qk_rot[lt][h]
                        ch = cosD[:, 2 * ls:2 * ls + 1024]
                        sh = sinD[:, 2 * ls:2 * ls + 1024]
                        t1 = wpool.tile([64, 1024], F16, name="t1", tag="t1",
                                        bufs=2)
                        t2 = wpool.tile([64, 1024], F16, name="t2", tag="t2",
                                        bufs=2)
                        rwhi = wpool.tile([64, 1024], F16, name="rwhi",
                                          tag="rwhi", bufs=2)
                        nc.vector.tensor_copy(out=rwhi[:], in_=rw[64:128, :])
                        nc.vector.tensor_mul(out=t1[:], in0=rw[0:64, :],
                                             in1=ch)
                        nc.vector.tensor_mul(out=t2[:], in0=rwhi[:], in1=sh)
                        nc.vector.tensor_sub(out=dst[0:64, :], in0=t1[:],
                                             in1=t2[:])
                        nc.vector.tensor_mul(out=t1[:], in0=rw[0:64, :],
                                             in1=sh)
                        nc.vector.tensor_mul(out=t2[:], in0=rwhi[:], in1=ch)
                        nc.vector.tensor_add(out=dst[64:128, :], in0=t1[:],
                                             in1=t2[:])
                    for h in range(2):
                        dst = qk_rot[lt][h]
                        nc.vector.tensor_mul(out=dst[:, :], in0=dst[:, :],
                                             in1=s2ps[h][:])

                return units, emit_rope

            def emit_vproj_units(lt):
                xb = xbig[lt]
                units = []

                def mkv(j):
                    state = {}

                    def u():
                        lb = 4 * lt + j
                        vp = ps.tile([128, QT], F32, name=f"vp{lb}",
                                     tag="pqk", bufs=3)
                        for i in range(16):
                            nc.tensor.matmul(
                                vp[:, 0:256],
                                xb[:, i * QT + j * 128:i * QT + (j + 1) * 128],
                                wv_all[:, i * 256:(i + 1) * 256],
                                start=(i == 0), stop=(i == 15),
                            )
                        nc.vector.tensor_copy(out=vnat[lb][:],
                                              in_=vp[:, 0:256])
                    return u
                for j in range(4):
                    units.append(mkv(j))
                return units

            def attn_blocks(qt, h, kbs):
                """Return [(A, B)] closures for the given k-blocks of
                (qt, h) plus state creation on first use."""
                key = (qt, h)
                blocks = []
                for kb in kbs:
                    ent = plan[qt][kb]
                    if ent[0] == "skip":
                        continue
                    nst, negr = _narrow(ent)
                    holder = {}

                    def A(kb=kb, ent=ent, nst=nst, negr=negr, holder=holder):
                        if key not in astate:
                            astate[key] = {
                                "oT": ps.tile([128, QT], F32,
                                              name=f"oT{qt}_{h}",
                                              tag="oTfo", bufs=2),
                                "acc": wpool.tile([128, QT], F32R,
                                                  name="sacc", tag="sacc",
                                                  bufs=2),
                                "n": 0,
                            }
                        st = ps.tile([128, QT], F32, name=f"sT{qt}_{h}_{kb}",
                                     tag="sT", bufs=3)
                        nc.tensor.matmul(
                            st[:, nst:],
                            qk_rot[kb // 4][h][:, QT + (kb % 4) * 128:
                                               QT + (kb % 4 + 1) * 128],
                            qk_rot[qt][h][:, nst:QT],
                            start=True, stop=True,
                        )
                        if ent[0] == "part":
                            _, bi, bs, bw, zs, zw = ent
                            if bw:
                                nc.vector.tensor_add(
                                    out=st[:, bs:bs + bw],
                                    in0=st[:, bs:bs + bw],
                                    in1=btiles[bi][:, 0:bw])
                            if negr is not None:
                                nc.vector.tensor_scalar_add(
                                    out=st[:, negr[0]:negr[0] + negr[1]],
                                    in0=st[:, negr[0]:negr[0] + negr[1]],
                                    scalar1=NEG)
                        pt = wpool.tile([128, QT], F16, name="pt", tag="pt",
                                        bufs=6)
                        nc.scalar.activation(pt[:, nst:], st[:, nst:], Exp,
                                             bias=ebc[:, :], scale=SCALE)
                        holder["pt"] = pt
                        holder["nst"] = nst

                    def B(kb=kb, holder=holder, last=False):
                        stt = astate[key]
                        pt, nst = holder["pt"], holder["nst"]
                        if stt["n"] == 0:
                            nc.vector.tensor_copy(out=stt["acc"][:], in_=pt[:])
                        else:
                            nc.vector.tensor_add(out=stt["acc"][:, nst:],
                                                 in0=stt["acc"][:, nst:],
                                                 in1=pt[:, nst:])
                        nc.tensor.matmul(
                            stt["oT"][:, nst:],
                            vnat[kb][:, h * HD:(h + 1) * HD],
                            pt[:, nst:],
                            start=(stt["n"] == 0), stop=last,
                        )
                        stt["n"] += 1
                    blocks.append((A, B))
                return blocks

            def attn_fin(qt, h):
                def F():
                    key = (qt, h)
                    stt = astate.pop(key)
                    sums = ps.tile([128, QT], F32, name=f"sums{qt}_{h}",
                                   tag="sT", bufs=3)
                    nc.tensor.matmul(sums[0:1, :], ones128[:], stt["acc"][:],
                                     start=True, stop=True)
                    rinv = wpool.tile([1, QT], F32, name="rinv", tag="rinv",
                                      bufs=2)
                    nc.vector.reciprocal_approx_fast(out=rinv[:],
                                                     in_=sums[0:1, :])
                    rb = wpool.tile([128, QT], F32, name="rb", tag="rb",
                                    bufs=2)
                    nc.gpsimd.partition_broadcast(rb[:], rinv[:])
                    nc.vector.tensor_mul(
                        out=aot[h][:, qt * QT:(qt + 1) * QT],
                        in0=stt["oT"][:], in1=rb[:],
                    )
                return F

            def outproj_units(lt, fo_tag):
                units = []

                def mk(j):
                    obb = {}

                    def u(jt_list):
                        lb = 4 * lt + j
                        if "t" not in obb:
                            obb["t"] = wpool.tile([128, 4 * QT], F16,
                                                  name="obb", tag="obb",
                                                  bufs=2)
                        for jt in jt_list:
                            fo = ps.tile([128, QT], F32, name=f"fo{lb}_{jt}",
                                         tag=fo_tag,
                                         bufs=3 if fo_tag == "pqk" else 2)
                            for h in range(2):
                                nc.tensor.matmul(
                                    fo[:],
                                    aot[h][:, lb * 128:(lb + 1) * 128],
                                    wout_all[:, h * D + jt * QT:
                                             h * D + (jt + 1) * QT],
                                    start=(h == 0), stop=(h == 1),
                                )
                            nc.vector.tensor_copy(
                                out=obb["t"][:, jt * QT:(jt + 1) * QT],
                                in_=fo[:])
                        if jt_list[-1] == 3:
                            nc.sync.dma_start(
                                out_d.ap()[lb * 128:(lb + 1) * 128, :],
                                obb["t"][:],
                            )
                    return u
                for j in range(4):
                    u = mk(j)
                    units.append(lambda u=u: u([0, 1]))
                    units.append(lambda u=u: u([2, 3]))
                return units

            def mark_last(blocks_h):
                """Wrap the last B of a head's final slab with last=True."""
                A, B = blocks_h[-1]
                blocks_h[-1] = (A, lambda B=B: B(last=True))
                return blocks_h

            # ================= main pipeline =================
            for it in range(NQT + 1):
                # ---- group 1: chains(it) + outproj(it-2) as PE-dense
                # fillers, woven with diag-attn(it-1) + finalize(it-1)
                work = []
                if it >= 1:
                    qt = it - 1
                    diag = list(range(4 * qt, 4 * qt + 4))
                    for h in range(2):
                        blocks = attn_blocks(qt, h, diag)
                        if blocks:
                            blocks = mark_last(blocks)
                        work.extend(_stagger(blocks, [attn_fin(qt, h)]))
                if it >= 1:
                    work.extend(outproj_units(
                        it - 1, "oTfo" if it <= NQT - 1 else "pqk"))
                fillers = []
                rope_fn = None
                if it < NQT:
                    fillers, rope_fn = emit_chains(it)
                _weave(fillers, work)

                # ---- rope for this l-tile
                if rope_fn is not None:
                    rope_fn()

                # ---- group 2: vproj(it) woven with prefix-attn(qt=it)
                if it < NQT:
                    work2 = []
                    prefix = list(range(0, 4 * it))
                    for h in range(2):
                        blocks = attn_blocks(it, h, prefix)
                        work2.extend(_stagger(blocks, []))
                    _weave(emit_vproj_units(it), work2)

    nc.finalize()
    return nc


def _rope_perm(h):
    """Row order within one head's 128 q/k features: odd indices then even."""
    base = h * HD
    return np.concatenate([np.arange(1, HD, 2), np.arange(0, HD, 2)]) + base


def _host_prep(x, W_qkv, W_out):
    xT = np.ascontiguousarray(x[0].T).astype(np.float16)   # [D, L]
    # xL: [128, NQT*16*512]: per l-tile, 16 d-slices side by side
    xL = np.empty((128, NQT * 16 * QT), np.float16)
    for lt in range(NQT):
        for i in range(16):
            xL[:, lt * 8192 + i * QT:lt * 8192 + (i + 1) * QT] = \
                xT[i * 128:(i + 1) * 128, lt * QT:(lt + 1) * QT]
    inv_freq = 1.0 / (ROPE_BASE ** (np.arange(0, HD, 2, dtype=np.float64) / HD))
    ang = np.arange(L, dtype=np.float64)[:, None] * inv_freq[None, :]
    cos64 = np.ascontiguousarray(np.cos(ang).T).astype(np.float16)  # [64, L]
    sin64 = np.ascontiguousarray(np.sin(ang).T).astype(np.float16)
    cosD = np.empty((64, 2 * L), np.float16)
    sinD = np.empty((64, 2 * L), np.float16)
    for lt in range(NQT):
        seg = slice(lt * QT, (lt + 1) * QT)
        cosD[:, 2 * lt * QT:2 * lt * QT + QT] = cos64[:, seg]
        cosD[:, 2 * lt * QT + QT:2 * (lt + 1) * QT] = cos64[:, seg]
        sinD[:, 2 * lt * QT:2 * lt * QT + QT] = sin64[:, seg]
        sinD[:, 2 * lt * QT + QT:2 * (lt + 1) * QT] = sin64[:, seg]

    per_core = []
    for d in range(N_CORES):
        h0 = HPD * d
        # chain order: q_h0 | k_h0 | q_h1 | k_h1
        cols = []
        for h in range(2):
            cols.append(_rope_perm(h0 + h))          # q rows
            cols.append(D + _rope_perm(h0 + h))      # k rows
        wqkT = W_qkv[np.concatenate([cols[0], cols[1], cols[2], cols[3]]),
                     :].T.astype(np.float16)         # [2048, 512]
        wqkH = np.empty((128, 16 * 512), np.float16)
        for i in range(16):
            wqkH[:, i * 512:(i + 1) * 512] = wqkT[i * 128:(i + 1) * 128, :]
        rows_v = 2 * D + np.arange(h0 * HD, (h0 + 2) * HD)
        wvT = W_qkv[rows_v, :].T.astype(np.float16)  # [2048, 256]
        wvH = np.empty((128, 16 * 256), np.float16)
        for i in range(16):
            wvH[:, i * 256:(i + 1) * 256] = wvT[i * 128:(i + 1) * 128, :]
        woutT = W_out[:, h0 * HD:(h0 + 2) * HD].T.astype(np.float16)  # [256,D]
        woutH = np.empty((128, 2 * D), np.float16)
        for h in range(2):
            woutH[:, h * D:(h + 1) * D] = woutT[h * 128:(h + 1) * 128, :]
        per_core.append((wqkH, wvH, woutH))
    return xL, cosD, sinD, per_core


def kernel(x, W_qkv, W_out, block_mask):
    x = np.asarray(x, dtype=np.float32)
    W_qkv = np.asarray(W_qkv, dtype=np.float32)
    W_out = np.asarray(W_out, dtype=np.float32)
    bm = np.asarray(block_mask).astype(bool)

    plan, biases = _classify_mask(bm)
    key = (plan, biases.shape[0])
    if key not in _prog_cache:
        _prog_cache[key] = _build_program(plan, biases.shape[0])
    nc = _prog_cache[key]

    xL, cosD, sinD, per_core = _host_prep(x, W_qkv, W_out)
    in_maps = []
    for d in range(N_CORES):
        wqkH, wvH, woutH = per_core[d]
        in_maps.append({
            "xL": xL, "wqkH": wqkH, "wvH": wvH, "woutH": woutH,
            "cosD": cosD, "sinD": sinD, "biasT": biases,
            "ones16": np.ones((128, 1), np.float16),
            "ones128": np.ones((128, 1), np.float32),
        })
    res = bass_utils.run_bass_kernel_spmd(nc, in_maps, list(range(N_CORES)))
    acc = np.zeros((L, D), np.float32)
    for r in res.results:
        acc += r["out"].astype(np.float32)
    return acc[None, :, :]


# revision 27
# speedup vs baseline: 1.0804x; 1.0804x over previous
"""Trainium2 Bass kernel for nn_MultiHeadAttention (fused QKV + RMS-norm +
RoPE + masked softmax attention + out-proj), tensor-parallel over heads
across 8 NeuronCores.

Contract: kernel(**inputs) takes FULL inputs, returns FULL output.
Self-contained: imports only numpy + the concourse framework.
"""
import numpy as np

import concourse.bacc as bacc
import concourse.mybir as mybir
import concourse.tile as tile
from concourse import bass_utils

N_HEAD = 16
ROPE_BASE = 100000.0
RMS_EPS = 1e-5
L = 2048
D = 2048
HD = 128          # head dim
N_CORES = 8
HPD = N_HEAD // N_CORES   # heads per device = 2
QT = 512          # q tile (attention + l-tile width)
NQT = L // QT     # 4
NKB = L // 128    # 16 k-blocks
NEG = -1.0e9
SCALE = 1.0 / np.sqrt(HD)
EXP_BIAS = -0.7   # keeps exp() outputs < fp16 max (score*SCALE <= 11.32)

F32 = mybir.dt.float32
F32R = mybir.dt.float32r
F16 = mybir.dt.float16

_prog_cache: dict = {}


def _classify_mask(bm: np.ndarray):
    """Per (q-tile, k-block) classification of the boolean mask.

    plan[qt][kb] = ('full',) | ('skip',) | ('part', bias_idx, bstart, bw,
    zstart, zw): add biases[bias_idx][:, :bw] to score cols
    [bstart:bstart+bw], add NEG to cols [zstart:zstart+zw]. biases is
    [n, 128, 512] fp32 (col-sliced bias subtiles, zero-padded)."""
    plan = []
    uniq = {}
    biases = []
    for qt in range(NQT):
        row = []
        for kb in range(NKB):
            reg = bm[qt * QT:(qt + 1) * QT, kb * 128:(kb + 1) * 128]
            if reg.all():
                row.append(("full",))
                continue
            if not reg.any():
                row.append(("skip",))
                continue
            regT = reg.T  # [128 k, 512 q]
            col_all = regT.all(axis=0)       # fully open columns
            col_none = (~regT).all(axis=0)   # fully masked columns
            mixed = ~(col_all | col_none)
            mix_idx = np.nonzero(mixed)[0]
            none_idx = np.nonzero(col_none)[0]
            # contiguity of ranges (true for causal masks)
            ok = True
            bs = bw = zs = zw = 0
            if mix_idx.size:
                bs, be = int(mix_idx[0]), int(mix_idx[-1]) + 1
                bw = be - bs
                ok &= bool(mixed[bs:be].all())
            if none_idx.size:
                zs, ze = int(none_idx[0]), int(none_idx[-1]) + 1
                zw = ze - zs
                ok &= bool(col_none[zs:ze].all())
                ok &= not (bw and not (ze <= bs or be <= zs))
            if not ok:
                # fallback: full-width bias
                bs, bw, zs, zw = 0, QT, 0, 0
            if bw:
                bias = np.zeros((128, QT), np.float32)
                bias[:, 0:bw] = np.where(regT[:, bs:bs + bw], np.float32(0),
                                         np.float32(NEG))
                key = (bw, bias.tobytes())
                if key not in uniq:
                    uniq[key] = len(biases)
                    biases.append(bias)
                bi = uniq[key]
            else:
                bi = 0
            row.append(("part", bi, bs, bw, zs, zw))
        plan.append(tuple(row))
    if not biases:
        biases.append(np.zeros((128, QT), np.float32))
    return tuple(plan), np.stack(biases)


def _narrow(ent):
    """Column range start + NEG fallback region for a plan entry."""
    if ent[0] != "part":
        return 0, None
    _, bi, bs, bw, zs, zw = ent
    if zw and zs == 0:
        return zw, None           # masked prefix: just skip those columns
    if zw:
        return 0, (zs, zw)        # masked suffix (non-causal fallback)
    return 0, None


def _stagger(blocks, tail):
    """Software-pipeline: emit A(n+1) before B(n) so the in-order PE
    stream never waits on block n's exp before issuing block n+1's MM."""
    seq = []
    prevB = None
    for A, B in blocks:
        seq.append(A)
        if prevB is not None:
            seq.append(prevB)
        prevB = B
    if prevB is not None:
        seq.append(prevB)
    seq.extend(tail)
    return seq


def _weave(fillers, work):
    """Interleave filler units (dense independent PE work) between work
    units so dependency stalls in `work` are hidden."""
    if not work:
        for f in fillers:
            f()
        return
    if not fillers:
        for w in work:
            w()
        return
    per = len(fillers) / len(work)
    fi = 0.0
    done = 0
    for w in work:
        tgt = fi + per
        while done < min(len(fillers), int(tgt + 0.5)):
            fillers[done]()
            done += 1
        fi = tgt
        w()
    while done < len(fillers):
        fillers[done]()
        done += 1


def _build_program(plan, n_bias):
    nc = bacc.Bacc("TRN2", target_bir_lowering=False, debug=False)

    # ---- DRAM I/O (host pre-arranged for wide DMA lines) ----
    # xL: per l-tile, 16 d-slices of [128, 512] concatenated along free dim.
    xL_d = nc.dram_tensor("xL", [128, NQT * 16 * QT], F16, kind="ExternalInput")
    # wqkH: 16 d-slices of [128, 512] (cols: q_h0 | k_h0 | q_h1 | k_h1)
    wqk_d = nc.dram_tensor("wqkH", [128, 16 * 512], F16, kind="ExternalInput")
    wv_d = nc.dram_tensor("wvH", [128, 16 * 256], F16, kind="ExternalInput")
    wout_d = nc.dram_tensor("woutH", [128, 2 * D], F16, kind="ExternalInput")
    # cosD/sinD: per l-tile slice duplicated (for paired q|k rope ops)
    cos_d = nc.dram_tensor("cosD", [64, 2 * L], F16, kind="ExternalInput")
    sin_d = nc.dram_tensor("sinD", [64, 2 * L], F16, kind="ExternalInput")
    ones16_d = nc.dram_tensor("ones16", [128, 1], F16, kind="ExternalInput")
    ones_d = nc.dram_tensor("ones128", [128, 1], F32R, kind="ExternalInput")
    bias_d = nc.dram_tensor("biasT", [n_bias, 128, QT], F32, kind="ExternalInput")
    out_d = nc.dram_tensor("out", [L, D], F16, kind="ExternalOutput")

    Exp = mybir.ActivationFunctionType.Exp
    Ln = mybir.ActivationFunctionType.Ln

    with nc.allow_low_precision(reason="fp16 activations; fp32 accumulation"), \
         tile.TileContext(nc) as tc:
        with (
            tc.tile_pool(name="const", bufs=1) as cpool,
            tc.tile_pool(name="act", bufs=1) as apool,
            tc.tile_pool(name="wrk", bufs=1) as wpool,
            tc.tile_pool(name="ps", bufs=1, space="PSUM") as ps,
        ):
            # ---- tiny consts first ----
            ones16 = cpool.tile([128, 1], F16, name="ones16", tag="ones16")
            nc.sync.dma_start(ones16[:], ones16_d.ap())
            epsc = cpool.tile([128, 1], F32, name="epsc", tag="epsc")
            nc.vector.memset(epsc[:], RMS_EPS)
            ebc = cpool.tile([128, 1], F32, name="ebc", tag="ebc")
            nc.vector.memset(ebc[:], EXP_BIAS)
            ones128 = cpool.tile([128, 1], F32R, name="ones128", tag="ones128")
            nc.sync.dma_start(ones128[:], ones_d.ap())

            # ---- bulk inputs: ALL issued before any output DMA so the
            # in-order sync queue never blocks a later input behind an
            # output waiting on compute. l-tile 0 x/weights interleaved in
            # 4-slice chunks so the first projection chains start early.
            xbig = [cpool.tile([128, 16 * QT], F16, name=f"xb{lt}",
                               tag=f"xb{lt}") for lt in range(NQT)]
            wqk_all = cpool.tile([128, 16 * 512], F16, name="wqk_all",
                                 tag="wqk_all")
            CH = 4 * QT  # 4-slice chunk
            for c in range(4):
                nc.sync.dma_start(xbig[0][:, c * CH:(c + 1) * CH],
                                  xL_d.ap()[:, c * CH:(c + 1) * CH])
                nc.sync.dma_start(wqk_all[:, c * 2048:(c + 1) * 2048],
                                  wqk_d.ap()[:, c * 2048:(c + 1) * 2048])
            cosD = cpool.tile([64, 2 * L], F16, name="cosD", tag="cosD")
            sinD = cpool.tile([64, 2 * L], F16, name="sinD", tag="sinD")
            nc.sync.dma_start(cosD[:], cos_d.ap())
            nc.sync.dma_start(sinD[:], sin_d.ap())
            nc.sync.dma_start(xbig[1][:], xL_d.ap()[:, 8192:2 * 8192])
            wv_all = cpool.tile([128, 16 * 256], F16, name="wv_all",
                                tag="wv_all")
            nc.sync.dma_start(wv_all[:], wv_d.ap())
            btiles = []
            for b in range(n_bias):
                bt = cpool.tile([128, QT], F32, name=f"bias{b}", tag=f"bias{b}")
                nc.sync.dma_start(bt[:], bias_d.ap()[b])
                btiles.append(bt)
            wout_all = cpool.tile([128, 2 * D], F16, name="wout_all",
                                  tag="wout_all")
            nc.sync.dma_start(wout_all[:], wout_d.ap())
            nc.sync.dma_start(xbig[2][:], xL_d.ap()[:, 2 * 8192:3 * 8192])
            nc.sync.dma_start(xbig[3][:], xL_d.ap()[:, 3 * 8192:4 * 8192])
            # Delay then trigger the ~9us gpsimd library load: a plain DMA
            # (no library needed) makes the in-order gpsimd engine wait for
            # l-tile 0's chunk-2 before it reaches the LOAD_LIB that the
            # framework inserts ahead of the first custom op (the dummy
            # broadcast).  The load then overlaps l-tile-0 compute instead
            # of stealing DMA bandwidth from the startup-critical fill.
            db_in = cpool.tile([1, 16], F16, name="db_in", tag="db_in")
            nc.gpsimd.dma_start(db_in[:], xbig[0][0:1, 6128:6144])
            db_out = cpool.tile([128, 16], F16, name="db_out", tag="db_out")
            nc.gpsimd.partition_broadcast(db_out[:], db_in[:])

            # ---- persistent activations ----
            # qk_rot[lt][h]: [128, 1024] fp16, cols 0:512 = q (rope+rms),
            # cols 512:1024 = k (rope+rms) -> doubles as the k cache.
            qk_rot = [[apool.tile([128, 1024], F16, name=f"qkr{lt}_{h}",
                                  tag=f"qkr{lt}_{h}") for h in range(HPD)]
                      for lt in range(NQT)]
            aot = [apool.tile([128, L], F16, name=f"aot{h}", tag=f"aot{h}")
                   for h in range(HPD)]
            vnat = [apool.tile([128, 2 * HD], F16, name=f"vnat{lb}",
                               tag=f"vnat{lb}") for lb in range(16)]

            # attention state per (qt, h), held across emission groups
            astate = {}

            def emit_chains(lt):
                """q/k projection chains -> filler unit closures (4 MMs each)
                plus trailing raw/sq/ssq/rms/rope emission attached."""
                xb = xbig[lt]
                s1s = [None] * 4
                rawp = [wpool.tile([128, 1024], F16, name="rawp", tag="rawp",
                                   bufs=4) for _ in range(2)]
                pps = []
                if lt == 0:
                    for ob in range(4):
                        tag, nb = (("pqk", 3) if ob < 2 else ("sT", 3))
                        pps.append(ps.tile([128, QT], F32, name=f"pqk0_{ob}",
                                           tag=tag, bufs=nb))
                else:
                    pps = [None] * 4

                units = []
                if lt == 0:
                    # DMA-inflow-bound: i-outer over 4 concurrent chains
                    def mk0(i):
                        def u():
                            for ob in range(4):
                                nc.tensor.matmul(
                                    pps[ob][:],
                                    wqk_all[:, i * 512 + ob * 128:
                                            i * 512 + (ob + 1) * 128],
                                    xb[:, i * QT:(i + 1) * QT],
                                    start=(i == 0), stop=(i == 15),
                                )
                            if i == 15:
                                for ob in range(4):
                                    _chain_tail(ob)
                        return u
                    units = [mk0(i) for i in range(16)]
                else:
                    def mkc(ob, ig):
                        def u():
                            if ig == 0:
                                pps[ob] = ps.tile(
                                    [128, QT], F32, name=f"pqk{lt}_{ob}",
                                    tag="pqk", bufs=3)
                            for i in range(4 * ig, 4 * ig + 4):
                                nc.tensor.matmul(
                                    pps[ob][:],
                                    wqk_all[:, i * 512 + ob * 128:
                                            i * 512 + (ob + 1) * 128],
                                    xb[:, i * QT:(i + 1) * QT],
                                    start=(i == 0), stop=(i == 15),
                                )
                            if ig == 3:
                                _chain_tail(ob)
                        return u
                    units = [mkc(ob, ig) for ob in range(4) for ig in range(4)]

                def _chain_tail(ob):
                    h, isk = ob // 2, ob % 2
                    raw = rawp[h][:, isk * QT:(isk + 1) * QT]
                    nc.vector.tensor_copy(out=raw, in_=pps[ob][:])
                    sq = wpool.tile([128, QT], F16, name="sq", tag="sq",
                                    bufs=2)
                    nc.vector.tensor_mul(out=sq[:], in0=raw, in1=raw)
                    ssq = ps.tile([128, QT], F32, name=f"ssq{lt}_{ob}",
                                  tag="sT", bufs=3)
                    nc.tensor.matmul(ssq[0:1, :], ones16[:], sq[:],
                                     start=True, stop=True)
                    # rms scale s = exp(-0.5*ln(ssq/HD + eps)) = rsqrt(mean)
                    lnm = wpool.tile([1, QT], F32, name="lnm", tag="lnm",
                                     bufs=2)
                    nc.scalar.activation(lnm[:], ssq[0:1, :], Ln,
                                         bias=epsc[0:1, :], scale=1.0 / HD)
                    s1 = wpool.tile([1, QT], F16, name="s1", tag="s1", bufs=4)
                    nc.scalar.activation(s1[:], lnm[:], Exp, scale=-0.5)
                    s1s[ob] = s1

                def emit_rope():
                    ls = lt * QT
                    s2ps = []
                    for h in range(2):
                        s2p = wpool.tile([128, 1024], F16, name="s2p",
                                         tag="s2p", bufs=2)
                        nc.gpsimd.partition_broadcast(s2p[:, 0:QT],
                                                      s1s[2 * h][:])
                        nc.gpsimd.partition_broadcast(s2p[:, QT:1024],
                                                      s1s[2 * h + 1][:])
                        s2ps.append(s2p)
                    # rope cores for both heads first (no s2p dependency),
                    # then the two rms-scale muls (which wait on gpsimd)
                    for h in range(2):
                        rw = rawp[h]
                        dst = qk_rot[lt][h]
                        ch = cosD[:, 2 * ls:2 * ls + 1024]
                        sh = sinD[:, 2 * ls:2 * ls + 1024]
                        t1 = wpool.tile([64, 1024], F16, name="t1", tag="t1",
                                        bufs=2)
                        t2 = wpool.tile([64, 1024], F16, name="t2", tag="t2",
                                        bufs=2)
                        rwhi = wpool.tile([64, 1024], F16, name="rwhi",
                                          tag="rwhi", bufs=2)
                        nc.vector.tensor_copy(out=rwhi[:], in_=rw[64:128, :])
                        nc.vector.tensor_mul(out=t1[:], in0=rw[0:64, :],
                                             in1=ch)
                        nc.vector.tensor_mul(out=t2[:], in0=rwhi[:], in1=sh)
                        nc.vector.tensor_sub(out=dst[0:64, :], in0=t1[:],
                                             in1=t2[:])
                        nc.vector.tensor_mul(out=t1[:], in0=rw[0:64, :],
                                             in1=sh)
                        nc.vector.tensor_mul(out=t2[:], in0=rwhi[:], in1=ch)
                        nc.vector.tensor_add(out=dst[64:128, :], in0=t1[:],
                                             in1=t2[:])
                    for h in range(2):
                        dst = qk_rot[lt][h]
                        nc.vector.tensor_mul(out=dst[:, :], in0=dst[:, :],
                                             in1=s2ps[h][:])

                return units, emit_rope

            def emit_vproj_units(lt):
                xb = xbig[lt]
                units = []

                def mkv(j):
                    state = {}

                    def u():
                        lb = 4 * lt + j
                        vp = ps.tile([128, QT], F32, name=f"vp{lb}",
                                     tag="pqk", bufs=3)
                        for i in range(16):
                            nc.tensor.matmul(
                                vp[:, 0:256],
                                xb[:, i * QT + j * 128:i * QT + (j + 1) * 128],
                                wv_all[:, i * 256:(i + 1) * 256],
                                start=(i == 0), stop=(i == 15),
                            )
                        nc.scalar.copy(vnat[lb][:], vp[:, 0:256])
                    return u
                for j in range(4):
                    units.append(mkv(j))
                return units

            def attn_blocks(qt, h, kbs):
                """Return [(A, B)] closures for the given k-blocks of
                (qt, h) plus state creation on first use."""
                key = (qt, h)
                blocks = []
                for kb in kbs:
                    ent = plan[qt][kb]
                    if ent[0] == "skip":
                        continue
                    nst, negr = _narrow(ent)
                    holder = {}

                    def A(kb=kb, ent=ent, nst=nst, negr=negr, holder=holder):
                        if key not in astate:
                            astate[key] = {
                                "oT": ps.tile([128, QT], F32,
                                              name=f"oT{qt}_{h}",
                                              tag="oTfo", bufs=2),
                                "acc": wpool.tile([128, QT], F32R,
                                                  name="sacc", tag="sacc",
                                                  bufs=2),
                                "n": 0,
                            }
                        st = ps.tile([128, QT], F32, name=f"sT{qt}_{h}_{kb}",
                                     tag="sT", bufs=3)
                        nc.tensor.matmul(
                            st[:, nst:],
                            qk_rot[kb // 4][h][:, QT + (kb % 4) * 128:
                                               QT + (kb % 4 + 1) * 128],
                            qk_rot[qt][h][:, nst:QT],
                            start=True, stop=True,
                        )
                        if ent[0] == "part":
                            _, bi, bs, bw, zs, zw = ent
                            if bw:
                                nc.vector.tensor_add(
                                    out=st[:, bs:bs + bw],
                                    in0=st[:, bs:bs + bw],
                                    in1=btiles[bi][:, 0:bw])
                            if negr is not None:
                                nc.vector.tensor_scalar_add(
                                    out=st[:, negr[0]:negr[0] + negr[1]],
                                    in0=st[:, negr[0]:negr[0] + negr[1]],
                                    scalar1=NEG)
                        pt = wpool.tile([128, QT], F16, name="pt", tag="pt",
                                        bufs=6)
                        nc.scalar.activation(pt[:, nst:], st[:, nst:], Exp,
                                             bias=ebc[:, :], scale=SCALE)
                        holder["pt"] = pt
                        holder["nst"] = nst

                    def B(kb=kb, holder=holder, last=False):
                        stt = astate[key]
                        pt, nst = holder["pt"], holder["nst"]
                        if stt["n"] == 0:
                            nc.vector.tensor_copy(out=stt["acc"][:], in_=pt[:])
                        else:
                            nc.vector.tensor_add(out=stt["acc"][:, nst:],
                                                 in0=stt["acc"][:, nst:],
                                                 in1=pt[:, nst:])
                        nc.tensor.matmul(
                            stt["oT"][:, nst:],
                            vnat[kb][:, h * HD:(h + 1) * HD],
                            pt[:, nst:],
                            start=(stt["n"] == 0), stop=last,
                        )
                        stt["n"] += 1
                    blocks.append((A, B))
                return blocks

            def attn_fin(qt, h):
                def F():
                    key = (qt, h)
                    stt = astate.pop(key)
                    sums = ps.tile([128, QT], F32, name=f"sums{qt}_{h}",
                                   tag="sT", bufs=3)
                    nc.tensor.matmul(sums[0:1, :], ones128[:], stt["acc"][:],
                                     start=True, stop=True)
                    rinv = wpool.tile([1, QT], F32, name="rinv", tag="rinv",
                                      bufs=2)
                    nc.vector.reciprocal_approx_fast(out=rinv[:],
                                                     in_=sums[0:1, :])
                    rb = wpool.tile([128, QT], F32, name="rb", tag="rb",
                                    bufs=2)
                    nc.gpsimd.partition_broadcast(rb[:], rinv[:])
                    nc.vector.tensor_mul(
                        out=aot[h][:, qt * QT:(qt + 1) * QT],
                        in0=stt["oT"][:], in1=rb[:],
                    )
                return F

            def outproj_units(lt, fo_tag):
                units = []

                def mk(j):
                    obb = {}

                    def u(jt_list):
                        lb = 4 * lt + j
                        if "t" not in obb:
                            obb["t"] = wpool.tile([128, 4 * QT], F16,
                                                  name="obb", tag="obb",
                                                  bufs=2)
                        for jt in jt_list:
                            fo = ps.tile([128, QT], F32, name=f"fo{lb}_{jt}",
                                         tag=fo_tag,
                                         bufs=3 if fo_tag == "pqk" else 2)
                            for h in range(2):
                                nc.tensor.matmul(
                                    fo[:],
                                    aot[h][:, lb * 128:(lb + 1) * 128],
                                    wout_all[:, h * D + jt * QT:
                                             h * D + (jt + 1) * QT],
                                    start=(h == 0), stop=(h == 1),
                                )
                            if (j + jt) % 2 == 0:
                                nc.vector.tensor_copy(
                                    out=obb["t"][:, jt * QT:(jt + 1) * QT],
                                    in_=fo[:])
                            else:
                                nc.scalar.copy(
                                    obb["t"][:, jt * QT:(jt + 1) * QT], fo[:])
                        if jt_list[-1] == 3:
                            nc.sync.dma_start(
                                out_d.ap()[lb * 128:(lb + 1) * 128, :],
                                obb["t"][:],
                            )
                    return u
                for j in range(4):
                    u = mk(j)
                    units.append(lambda u=u: u([0, 1]))
                    units.append(lambda u=u: u([2, 3]))
                return units

            def mark_last(blocks_h):
                """Wrap the last B of a head's final slab with last=True."""
                A, B = blocks_h[-1]
                blocks_h[-1] = (A, lambda B=B: B(last=True))
                return blocks_h

            # ================= main pipeline =================
            for it in range(NQT + 1):
                # ---- group 1: chains(it) + outproj(it-2) as PE-dense
                # fillers, woven with diag-attn(it-1) + finalize(it-1)
                work = []
                if it >= 1:
                    qt = it - 1
                    diag = list(range(4 * qt, 4 * qt + 4))
                    for h in range(2):
                        blocks = attn_blocks(qt, h, diag)
                        if blocks:
                            blocks = mark_last(blocks)
                        work.extend(_stagger(blocks, [attn_fin(qt, h)]))
                if it >= 1:
                    work.extend(outproj_units(
                        it - 1, "oTfo" if it <= NQT - 1 else "pqk"))
                fillers = []
                rope_fn = None
                if it < NQT:
                    fillers, rope_fn = emit_chains(it)
                _weave(fillers, work)

                # ---- rope for this l-tile
                if rope_fn is not None:
                    rope_fn()

                # ---- group 2: vproj(it) woven with prefix-attn(qt=it)
                if it < NQT:
                    work2 = []
                    prefix = list(range(0, 4 * it))
                    for h in range(2):
                        blocks = attn_blocks(it, h, prefix)
                        work2.extend(_stagger(blocks, []))
                    _weave(emit_vproj_units(it), work2)

    nc.finalize()
    return nc


def _rope_perm(h):
    """Row order within one head's 128 q/k features: odd indices then even."""
    base = h * HD
    return np.concatenate([np.arange(1, HD, 2), np.arange(0, HD, 2)]) + base


def _host_prep(x, W_qkv, W_out):
    xT = np.ascontiguousarray(x[0].T).astype(np.float16)   # [D, L]
    # xL: [128, NQT*16*512]: per l-tile, 16 d-slices side by side
    xL = np.empty((128, NQT * 16 * QT), np.float16)
    for lt in range(NQT):
        for i in range(16):
            xL[:, lt * 8192 + i * QT:lt * 8192 + (i + 1) * QT] = \
                xT[i * 128:(i + 1) * 128, lt * QT:(lt + 1) * QT]
    inv_freq = 1.0 / (ROPE_BASE ** (np.arange(0, HD, 2, dtype=np.float64) / HD))
    ang = np.arange(L, dtype=np.float64)[:, None] * inv_freq[None, :]
    cos64 = np.ascontiguousarray(np.cos(ang).T).astype(np.float16)  # [64, L]
    sin64 = np.ascontiguousarray(np.sin(ang).T).astype(np.float16)
    cosD = np.empty((64, 2 * L), np.float16)
    sinD = np.empty((64, 2 * L), np.float16)
    for lt in range(NQT):
        seg = slice(lt * QT, (lt + 1) * QT)
        cosD[:, 2 * lt * QT:2 * lt * QT + QT] = cos64[:, seg]
        cosD[:, 2 * lt * QT + QT:2 * (lt + 1) * QT] = cos64[:, seg]
        sinD[:, 2 * lt * QT:2 * lt * QT + QT] = sin64[:, seg]
        sinD[:, 2 * lt * QT + QT:2 * (lt + 1) * QT] = sin64[:, seg]

    per_core = []
    for d in range(N_CORES):
        h0 = HPD * d
        # chain order: q_h0 | k_h0 | q_h1 | k_h1
        cols = []
        for h in range(2):
            cols.append(_rope_perm(h0 + h))          # q rows
            cols.append(D + _rope_perm(h0 + h))      # k rows
        wqkT = W_qkv[np.concatenate([cols[0], cols[1], cols[2], cols[3]]),
                     :].T.astype(np.float16)         # [2048, 512]
        wqkH = np.empty((128, 16 * 512), np.float16)
        for i in range(16):
            wqkH[:, i * 512:(i + 1) * 512] = wqkT[i * 128:(i + 1) * 128, :]
        rows_v = 2 * D + np.arange(h0 * HD, (h0 + 2) * HD)
        wvT = W_qkv[rows_v, :].T.astype(np.float16)  # [2048, 256]
        wvH = np.empty((128, 16 * 256), np.float16)
        for i in range(16):
            wvH[:, i * 256:(i + 1) * 256] = wvT[i * 128:(i + 1) * 128, :]
        woutT = W_out[:, h0 * HD:(h0 + 2) * HD].T.astype(np.float16)  # [256,D]
        woutH = np.empty((128, 2 * D), np.float16)
        for h in range(2):
            woutH[:, h * D:(h + 1) * D] = woutT[h * 128:(h + 1) * 128, :]
        per_core.append((wqkH, wvH, woutH))
    return xL, cosD, sinD, per_core


def kernel(x, W_qkv, W_out, block_mask):
    x = np.asarray(x, dtype=np.float32)
    W_qkv = np.asarray(W_qkv, dtype=np.float32)
    W_out = np.asarray(W_out, dtype=np.float32)
    bm = np.asarray(block_mask).astype(bool)

    plan, biases = _classify_mask(bm)
    key = (plan, biases.shape[0])
    if key not in _prog_cache:
        _prog_cache[key] = _build_program(plan, biases.shape[0])
    nc = _prog_cache[key]

    xL, cosD, sinD, per_core = _host_prep(x, W_qkv, W_out)
    in_maps = []
    for d in range(N_CORES):
        wqkH, wvH, woutH = per_core[d]
        in_maps.append({
            "xL": xL, "wqkH": wqkH, "wvH": wvH, "woutH": woutH,
            "cosD": cosD, "sinD": sinD, "biasT": biases,
            "ones16": np.ones((128, 1), np.float16),
            "ones128": np.ones((128, 1), np.float32),
        })
    res = bass_utils.run_bass_kernel_spmd(nc, in_maps, list(range(N_CORES)))
    acc = np.zeros((L, D), np.float32)
    for r in res.results:
        acc += r["out"].astype(np.float32)
    return acc[None, :, :]
